# revision 2
# baseline (speedup 1.0000x reference)
"""Contrastive-loss kernel for Trainium2 (8 NeuronCores, Bass/Tile).

Math (reference):
    W = wsi[:, 0, :], O = omic[:, 0, :]                      # [N, D]
    S = (W @ O.T) / max(|W_i||O_j|, eps)                     # [N, N] cosine sims
    d = diag(S)
    L = where(eye, 1 - S, relu(M - S + d[:, None]))
    out = mean(L)

Identity used on device: relu(M - S_ii + d_i) == M exactly, so
    sum(L) = sum_{i,j} relu(M - S_ij + d_i) + sum_i (1 - d_i - M)
which needs no diagonal masking of the big [N, N] block.

Distribution: data-parallel over W rows. Each core c gets its 512 W rows
(pre-normalized, pre-transposed to [D, rows] layout, bf16) plus the full
normalized O, transposed and column-rotated by 512*c so the diagonal block
always lands in j-chunk 0 (keeps the SPMD program core-independent). Each
core computes its [512, 4096] block of S on the PE (bf16 in, fp32 psum),
applies the hinge + row-sum on the Scalar engine, and DMAs out a [128, 1]
partial sum. Host adds the 8 partials and divides by N^2.
"""

import numpy as np
import ml_dtypes

N = 4096
D = 1024
NCORES = 8
ROWS = N // NCORES  # 512 W rows per core
P = 128             # SBUF partitions
NJ = 512            # moving free dim per matmul (one PSUM bank of fp32)
TI = ROWS // P      # 4 i-tiles per core
ND = D // P         # 8 contraction chunks
NJC = N // NJ       # 8 j-chunks
MARGIN = 0.1

_cache = {}


def _build():
    from contextlib import ExitStack
    import concourse.bacc as bacc
    import concourse.tile as tile
    import concourse.mybir as mybir

    f32 = mybir.dt.float32
    bf16 = mybir.dt.bfloat16

    nc = bacc.Bacc("TRN2", target_bir_lowering=False, debug=False,
                   num_devices=NCORES)
    wt_d = nc.dram_tensor("wt", [P, TI * ND * P], bf16, kind="ExternalInput").ap()
    ot_d = nc.dram_tensor("ot", [P, NJC * ND * NJ], bf16, kind="ExternalInput").ap()
    id_d = nc.dram_tensor("id", [P, P], f32, kind="ExternalInput").ap()
    out_d = nc.dram_tensor("out", [P, 1], f32, kind="ExternalOutput").ap()

    with tile.TileContext(nc) as tc, ExitStack() as ctx:
        const = ctx.enter_context(tc.tile_pool(name="const", bufs=1))
        otp = ctx.enter_context(tc.tile_pool(name="otp", bufs=NJC))
        pp = ctx.enter_context(tc.tile_pool(name="pp", bufs=4, space="PSUM"))
        scrp = ctx.enter_context(tc.tile_pool(name="scr", bufs=4))
        smallp = ctx.enter_context(tc.tile_pool(name="small", bufs=2))

        wt_sb = const.tile([P, TI * ND * P], bf16, tag="wt")
        nc.sync.dma_start(out=wt_sb[:], in_=wt_d[:, :])
        id_sb = const.tile([P, P], f32, tag="id")
        nc.sync.dma_start(out=id_sb[:], in_=id_d[:, :])
        ots = []
        for jc in range(NJC):
            o = otp.tile([P, ND * NJ], bf16, tag="ot")
            nc.sync.dma_start(out=o[:], in_=ot_d[:, jc * ND * NJ:(jc + 1) * ND * NJ])
            ots.append(o)

        # per-(t,jc) hinge row-sums + per-t corrections, reduced at the end
        acc = const.tile([P, TI * (NJC + 1)], f32, tag="acc")
        hb = const.tile([P, TI], f32, tag="hb")  # hb[:, t] = MARGIN + d_i

        for jc in range(NJC):
            for t in range(TI):
                ps = pp.tile([P, NJ], f32, tag="ps")
                for d in range(ND):
                    nc.tensor.matmul(
                        ps[:],
                        lhsT=wt_sb[:, (t * ND + d) * P:(t * ND + d + 1) * P],
                        rhs=ots[jc][:, d * NJ:(d + 1) * NJ],
                        start=(d == 0),
                        stop=(d == ND - 1),
                    )
                if jc == 0:
                    # hb[:, t] = MARGIN + d_i  (diag of this block via identity
                    # mask; tensor_tensor_reduce faults the exec unit on this
                    # runtime, so use mul + reduce + bias-add instead)
                    dprod = scrp.tile([P, P], f32, tag="dprod")
                    nc.vector.tensor_mul(
                        dprod[:], ps[:, t * P:(t + 1) * P], id_sb[:])
                    dsum = scrp.tile([P, 1], f32, tag="dsum")
                    nc.vector.tensor_reduce(
                        out=dsum[:], in_=dprod[:],
                        axis=mybir.AxisListType.X, op=mybir.AluOpType.add)
                    nc.scalar.activation(
                        out=hb[:, t:t + 1], in_=dsum[:],
                        func=mybir.ActivationFunctionType.Copy,
                        bias=MARGIN, scale=1.0)
                    # correction column: 1 - d_i - MARGIN = 1 - hb
                    nc.scalar.activation(
                        out=acc[:, t * (NJC + 1) + NJC:t * (NJC + 1) + NJC + 1],
                        in_=hb[:, t:t + 1],
                        func=mybir.ActivationFunctionType.Copy,
                        bias=1.0,
                        scale=-1.0,
                    )
                h = scrp.tile([P, NJ], f32, tag="h")
                nc.scalar.activation(
                    out=h[:],
                    in_=ps[:],
                    func=mybir.ActivationFunctionType.Relu,
                    bias=hb[:, t:t + 1],
                    scale=-1.0,
                    accum_out=acc[:, t * (NJC + 1) + jc:t * (NJC + 1) + jc + 1],
                )

        total = smallp.tile([P, 1], f32, tag="tot")
        nc.vector.tensor_reduce(
            out=total[:], in_=acc[:, :], axis=mybir.AxisListType.X,
            op=mybir.AluOpType.add,
        )
        nc.sync.dma_start(out=out_d[:, :], in_=total[:])

    nc.compile()
    return nc


def _get_nc():
    if "nc" not in _cache:
        _cache["nc"] = _build()
    return _cache["nc"]


def _prep_inputs(wsi, omic):
    W = np.asarray(wsi, dtype=np.float32)[:, 0, :].astype(np.float64)
    O = np.asarray(omic, dtype=np.float32)[:, 0, :].astype(np.float64)
    Wn = (W / np.maximum(np.linalg.norm(W, axis=1, keepdims=True), 1e-30))
    On = (O / np.maximum(np.linalg.norm(O, axis=1, keepdims=True), 1e-30))
    Wn = Wn.astype(ml_dtypes.bfloat16)
    On = On.astype(ml_dtypes.bfloat16)
    ident = np.eye(P, dtype=np.float32)

    in_maps = []
    for c in range(NCORES):
        Wc = Wn[c * ROWS:(c + 1) * ROWS]  # [512, 1024]
        # wt[k, (t*ND + d)*P + m] = Wc[t*P + m, d*P + k]
        wt = np.ascontiguousarray(
            Wc.reshape(TI, P, ND, P).transpose(3, 0, 2, 1).reshape(P, TI * ND * P))
        # column rotation: permuted col j' <-> original O row (j' + 512c) % N
        Operm = np.roll(On, -ROWS * c, axis=0)
        # ot[k, (jc*ND + d)*NJ + n] = Operm[jc*NJ + n, d*P + k]
        ot = np.ascontiguousarray(
            Operm.reshape(NJC, NJ, ND, P).transpose(3, 0, 2, 1)
            .reshape(P, NJC * ND * NJ))
        in_maps.append({"wt": wt, "ot": ot, "id": ident})
    return in_maps


def kernel(wsi_embeddings, omic_embeddings):
    from concourse.bass_utils import run_bass_kernel_spmd

    nc = _get_nc()
    in_maps = _prep_inputs(wsi_embeddings, omic_embeddings)
    res = run_bass_kernel_spmd(nc, in_maps, list(range(NCORES)))
    grand = 0.0
    for c in range(NCORES):
        grand += res.results[c]["out"][:, 0].astype(np.float64).sum()
    return np.float32(grand / (float(N) * float(N)))


# revision 6
# speedup vs baseline: 1.0766x; 1.0766x over previous
"""Contrastive-loss kernel for Trainium2 (8 NeuronCores, Bass/Tile).

Math (reference):
    W = wsi[:, 0, :], O = omic[:, 0, :]                      # [N, D]
    S = (W @ O.T) / max(|W_i||O_j|, eps)                     # [N, N] cosine sims
    d = diag(S)
    L = where(eye, 1 - S, relu(M - S + d[:, None]))
    out = mean(L)

Identity used on device: relu(M - S_ii + d_i) == M exactly, so
    sum(L) = sum_{i,j} relu(M - S_ij + d_i) + sum_i (1 - d_i - M)
which needs no diagonal masking of the big [N, N] block.

Distribution: data-parallel over W rows. Each core c gets its 512 W rows
(pre-normalized, pre-transposed to [D, rows] layout, bf16) plus the full
normalized O, transposed and column-rotated by 512*c so the diagonal block
always lands in j-chunk 0 (keeps the SPMD program core-independent). Each
core computes its [512, 4096] block of S on the PE (bf16 in, fp32 psum),
applies the hinge + row-sum on the Scalar engine, and DMAs out a [128, 1]
partial sum. Host adds the 8 partials and divides by N^2.
"""

import numpy as np
import ml_dtypes

N = 4096
D = 1024
NCORES = 8
ROWS = N // NCORES  # 512 W rows per core
P = 128             # SBUF partitions
NJ = 512            # moving free dim per matmul (one PSUM bank of fp32)
TI = ROWS // P      # 4 i-tiles per core
ND = D // P         # 8 contraction chunks
NJC = N // NJ       # 8 j-chunks
MARGIN = 0.1

_cache = {}


def _build():
    from contextlib import ExitStack
    import concourse.bacc as bacc
    import concourse.tile as tile
    import concourse.mybir as mybir

    f32 = mybir.dt.float32
    bf16 = mybir.dt.bfloat16

    nc = bacc.Bacc("TRN2", target_bir_lowering=False, debug=False,
                   num_devices=NCORES)
    wt_d = nc.dram_tensor("wt", [P, TI * ND * P], bf16, kind="ExternalInput").ap()
    ot_d = nc.dram_tensor("ot", [P, NJC * ND * NJ], bf16, kind="ExternalInput").ap()
    id_d = nc.dram_tensor("id", [P, P], f32, kind="ExternalInput").ap()
    out_d = nc.dram_tensor("out", [1, TI * (NJC + 1)], f32,
                           kind="ExternalOutput").ap()

    with tile.TileContext(nc) as tc, ExitStack() as ctx:
        const = ctx.enter_context(tc.tile_pool(name="const", bufs=1))
        otp = ctx.enter_context(tc.tile_pool(name="otp", bufs=NJC))
        pp = ctx.enter_context(tc.tile_pool(name="pp", bufs=4, space="PSUM"))
        scrp = ctx.enter_context(tc.tile_pool(name="scr", bufs=4))
        smallp = ctx.enter_context(tc.tile_pool(name="small", bufs=2))

        # DMA order puts the first matmul's operands (t=0 weights + j-chunk 0)
        # at the head of the HWDGE queue; everything else streams behind.
        wt_sb = const.tile([P, TI * ND * P], bf16, tag="wt")
        nc.sync.dma_start(out=wt_sb[:, 0:ND * P], in_=wt_d[:, 0:ND * P])
        ots = []
        o0 = otp.tile([P, ND * NJ], bf16, tag="ot")
        nc.sync.dma_start(out=o0[:], in_=ot_d[:, 0:ND * NJ])
        ots.append(o0)
        nc.sync.dma_start(out=wt_sb[:, ND * P:], in_=wt_d[:, ND * P:])
        id_sb = const.tile([P, P], f32, tag="id")
        nc.sync.dma_start(out=id_sb[:], in_=id_d[:, :])
        for jc in range(1, NJC):
            o = otp.tile([P, ND * NJ], bf16, tag="ot")
            nc.sync.dma_start(out=o[:], in_=ot_d[:, jc * ND * NJ:(jc + 1) * ND * NJ])
            ots.append(o)
        ones_sb = const.tile([P, 1], f32, tag="ones")
        nc.vector.memset(ones_sb[:], 1.0)

        # per-(t,jc) hinge row-sums + per-t corrections, reduced at the end
        acc = const.tile([P, TI * (NJC + 1)], f32, tag="acc")
        hb = const.tile([P, TI], f32, tag="hb")  # hb[:, t] = MARGIN + d_i

        for jc in range(NJC):
            for t in range(TI):
                ps = pp.tile([P, NJ], f32, tag="ps")
                for d in range(ND):
                    nc.tensor.matmul(
                        ps[:],
                        lhsT=wt_sb[:, (t * ND + d) * P:(t * ND + d + 1) * P],
                        rhs=ots[jc][:, d * NJ:(d + 1) * NJ],
                        start=(d == 0),
                        stop=(d == ND - 1),
                    )
                if jc == 0:
                    # hb[:, t] = MARGIN + d_i  (diag of this block via identity
                    # mask; tensor_tensor_reduce faults the exec unit on this
                    # runtime, so use mul + reduce + bias-add instead)
                    dprod = scrp.tile([P, P], f32, tag="dprod")
                    nc.vector.tensor_mul(
                        dprod[:], ps[:, t * P:(t + 1) * P], id_sb[:])
                    dsum = scrp.tile([P, 1], f32, tag="dsum")
                    nc.vector.tensor_reduce(
                        out=dsum[:], in_=dprod[:],
                        axis=mybir.AxisListType.X, op=mybir.AluOpType.add)
                    nc.scalar.activation(
                        out=hb[:, t:t + 1], in_=dsum[:],
                        func=mybir.ActivationFunctionType.Copy,
                        bias=MARGIN, scale=1.0)
                    # correction column: 1 - d_i - MARGIN = 1 - hb
                    nc.scalar.activation(
                        out=acc[:, t * (NJC + 1) + NJC:t * (NJC + 1) + NJC + 1],
                        in_=hb[:, t:t + 1],
                        func=mybir.ActivationFunctionType.Copy,
                        bias=1.0,
                        scale=-1.0,
                    )
                h = scrp.tile([P, NJ], f32, tag="h")
                nc.scalar.activation(
                    out=h[:],
                    in_=ps[:],
                    func=mybir.ActivationFunctionType.Relu,
                    bias=hb[:, t:t + 1],
                    scale=-1.0,
                    accum_out=acc[:, t * (NJC + 1) + jc:t * (NJC + 1) + jc + 1],
                )

        # cross-partition reduce on the PE (ones^T @ acc -> [1, 36]) so the
        # output DMA is one contiguous partition line instead of 128 4-byte
        # descriptors (whose completion receipts dominate the kernel tail)
        tot_ps = pp.tile([1, TI * (NJC + 1)], f32, tag="totps")
        nc.tensor.matmul(tot_ps[:], lhsT=ones_sb[:], rhs=acc[:, :],
                         start=True, stop=True)
        total = smallp.tile([1, TI * (NJC + 1)], f32, tag="tot")
        nc.vector.tensor_copy(total[:], tot_ps[:])
        nc.sync.dma_start(out=out_d[:, :], in_=total[:])

    nc.compile()
    return nc


def _get_nc():
    if "nc" not in _cache:
        _cache["nc"] = _build()
    return _cache["nc"]


def _prep_inputs(wsi, omic):
    W = np.asarray(wsi, dtype=np.float32)[:, 0, :].astype(np.float64)
    O = np.asarray(omic, dtype=np.float32)[:, 0, :].astype(np.float64)
    Wn = (W / np.maximum(np.linalg.norm(W, axis=1, keepdims=True), 1e-30))
    On = (O / np.maximum(np.linalg.norm(O, axis=1, keepdims=True), 1e-30))
    Wn = Wn.astype(ml_dtypes.bfloat16)
    On = On.astype(ml_dtypes.bfloat16)
    ident = np.eye(P, dtype=np.float32)

    in_maps = []
    for c in range(NCORES):
        Wc = Wn[c * ROWS:(c + 1) * ROWS]  # [512, 1024]
        # wt[k, (t*ND + d)*P + m] = Wc[t*P + m, d*P + k]
        wt = np.ascontiguousarray(
            Wc.reshape(TI, P, ND, P).transpose(3, 0, 2, 1).reshape(P, TI * ND * P))
        # column rotation: permuted col j' <-> original O row (j' + 512c) % N
        Operm = np.roll(On, -ROWS * c, axis=0)
        # ot[k, (jc*ND + d)*NJ + n] = Operm[jc*NJ + n, d*P + k]
        ot = np.ascontiguousarray(
            Operm.reshape(NJC, NJ, ND, P).transpose(3, 0, 2, 1)
            .reshape(P, NJC * ND * NJ))
        in_maps.append({"wt": wt, "ot": ot, "id": ident})
    return in_maps


def kernel(wsi_embeddings, omic_embeddings):
    from concourse.bass_utils import run_bass_kernel_spmd

    nc = _get_nc()
    in_maps = _prep_inputs(wsi_embeddings, omic_embeddings)
    res = run_bass_kernel_spmd(nc, in_maps, list(range(NCORES)))
    grand = 0.0
    for c in range(NCORES):
        grand += res.results[c]["out"].astype(np.float64).sum()
    return np.float32(grand / (float(N) * float(N)))


# revision 10
# speedup vs baseline: 1.0921x; 1.0144x over previous
"""Contrastive-loss kernel for Trainium2 (8 NeuronCores, Bass/Tile).

Math (reference):
    W = wsi[:, 0, :], O = omic[:, 0, :]                      # [N, D]
    S = (W @ O.T) / max(|W_i||O_j|, eps)                     # [N, N] cosine sims
    d = diag(S)
    L = where(eye, 1 - S, relu(M - S + d[:, None]))
    out = mean(L)

Identity used on device: relu(M - S_ii + d_i) == M exactly, so
    sum(L) = sum_{i,j} relu(M - S_ij + d_i) + sum_i (1 - d_i - M)
which needs no diagonal masking of the big [N, N] block.

Distribution: data-parallel over W rows. Each core c gets its 512 W rows
(pre-normalized, pre-transposed to [D, rows] layout, bf16) plus the full
normalized O, transposed and column-rotated by 512*c so the diagonal block
always lands in j-chunk 0 (keeps the SPMD program core-independent). Each
core computes its [512, 4096] block of S on the PE (bf16 in, fp32 psum),
applies the hinge + row-sum on the Scalar engine, and DMAs out a [128, 1]
partial sum. Host adds the 8 partials and divides by N^2.
"""

import numpy as np
import ml_dtypes

N = 4096
D = 1024
NCORES = 8
ROWS = N // NCORES  # 512 W rows per core
P = 128             # SBUF partitions
NJ = 512            # moving free dim per matmul (one PSUM bank of fp32)
TI = ROWS // P      # 4 i-tiles per core
ND = D // P         # 8 contraction chunks
NJC = N // NJ       # 8 j-chunks
MARGIN = 0.1
N_WARMUP = 100      # PE-warmup dummy matmuls issued while DMAs stream

_cache = {}


def _build():
    from contextlib import ExitStack
    import concourse.bacc as bacc
    import concourse.tile as tile
    import concourse.mybir as mybir

    f32 = mybir.dt.float32
    bf16 = mybir.dt.bfloat16

    nc = bacc.Bacc("TRN2", target_bir_lowering=False, debug=False,
                   num_devices=NCORES)
    wt_d = nc.dram_tensor("wt", [P, TI * ND * P], bf16, kind="ExternalInput").ap()
    ot_d = nc.dram_tensor("ot", [P, NJC * ND * NJ], bf16, kind="ExternalInput").ap()
    id_d = nc.dram_tensor("id", [P, P], f32, kind="ExternalInput").ap()
    out_d = nc.dram_tensor("out", [1, TI * (NJC + 1)], f32,
                           kind="ExternalOutput").ap()

    with tile.TileContext(nc) as tc, ExitStack() as ctx:
        const = ctx.enter_context(tc.tile_pool(name="const", bufs=1))
        otp = ctx.enter_context(tc.tile_pool(name="otp", bufs=NJC))
        pp = ctx.enter_context(tc.tile_pool(name="pp", bufs=4, space="PSUM"))
        pp1 = ctx.enter_context(tc.tile_pool(name="pp1", bufs=1, space="PSUM"))
        scrp = ctx.enter_context(tc.tile_pool(name="scr", bufs=4))
        smallp = ctx.enter_context(tc.tile_pool(name="small", bufs=2))

        # DMA order puts the first matmul's operands (t=0 weights + j-chunk 0)
        # at the head of the HWDGE queue; everything else streams behind.
        wt_sb = const.tile([P, TI * ND * P], bf16, tag="wt")
        nc.sync.dma_start(out=wt_sb[:, 0:ND * P], in_=wt_d[:, 0:ND * P])
        ots = []
        o0 = otp.tile([P, ND * NJ], bf16, tag="ot")
        nc.sync.dma_start(out=o0[:], in_=ot_d[:, 0:ND * NJ])
        ots.append(o0)
        nc.sync.dma_start(out=wt_sb[:, ND * P:], in_=wt_d[:, ND * P:])
        id_sb = const.tile([P, P], f32, tag="id")
        nc.sync.dma_start(out=id_sb[:], in_=id_d[:, :])
        for jc in range(1, NJC):
            o = otp.tile([P, ND * NJ], bf16, tag="ot")
            nc.sync.dma_start(out=o[:], in_=ot_d[:, jc * ND * NJ:(jc + 1) * ND * NJ])
            ots.append(o)
        ones_sb = const.tile([P, 1], f32, tag="ones")
        nc.vector.memset(ones_sb[:], 1.0)

        # Warm the PE clock (HAM gate releases after ~3.4us of activity)
        # while the first DMAs stream: tiny N=1 matmuls on the ones tile.
        warm_ps = pp1.tile([1, 1], f32, tag="warmps")
        for _ in range(N_WARMUP):
            nc.tensor.matmul(warm_ps[:], lhsT=ones_sb[:], rhs=ones_sb[:],
                             start=True, stop=True)

        # per-(t,jc) hinge row-sums + per-t corrections, reduced at the end
        acc = const.tile([P, TI * (NJC + 1)], f32, tag="acc")
        hb = const.tile([P, TI], f32, tag="hb")  # hb[:, t] = MARGIN + d_i

        for jc in range(NJC):
            for t in range(TI):
                ps = pp.tile([P, NJ], f32, tag="ps")
                for d in range(ND):
                    nc.tensor.matmul(
                        ps[:],
                        lhsT=wt_sb[:, (t * ND + d) * P:(t * ND + d + 1) * P],
                        rhs=ots[jc][:, d * NJ:(d + 1) * NJ],
                        start=(d == 0),
                        stop=(d == ND - 1),
                    )
                if jc == 0:
                    # hb[:, t] = MARGIN + d_i  (diag of this block via identity
                    # mask; tensor_tensor_reduce faults the exec unit on this
                    # runtime, so use mul + reduce + bias-add instead)
                    dprod = scrp.tile([P, P], f32, tag="dprod")
                    nc.vector.tensor_mul(
                        dprod[:], ps[:, t * P:(t + 1) * P], id_sb[:])
                    dsum = scrp.tile([P, 1], f32, tag="dsum")
                    nc.vector.tensor_reduce(
                        out=dsum[:], in_=dprod[:],
                        axis=mybir.AxisListType.X, op=mybir.AluOpType.add)
                    nc.scalar.activation(
                        out=hb[:, t:t + 1], in_=dsum[:],
                        func=mybir.ActivationFunctionType.Copy,
                        bias=MARGIN, scale=1.0)
                    # correction column: 1 - d_i - MARGIN = 1 - hb
                    nc.scalar.activation(
                        out=acc[:, t * (NJC + 1) + NJC:t * (NJC + 1) + NJC + 1],
                        in_=hb[:, t:t + 1],
                        func=mybir.ActivationFunctionType.Copy,
                        bias=1.0,
                        scale=-1.0,
                    )
                h = scrp.tile([P, NJ], f32, tag="h")
                nc.scalar.activation(
                    out=h[:],
                    in_=ps[:],
                    func=mybir.ActivationFunctionType.Relu,
                    bias=hb[:, t:t + 1],
                    scale=-1.0,
                    accum_out=acc[:, t * (NJC + 1) + jc:t * (NJC + 1) + jc + 1],
                )

        # cross-partition reduce on the PE (ones^T @ acc -> [1, 36]) so the
        # output DMA is one contiguous partition line instead of 128 4-byte
        # descriptors (whose completion receipts dominate the kernel tail)
        tot_ps = pp1.tile([1, TI * (NJC + 1)], f32, tag="totps")
        nc.tensor.matmul(tot_ps[:], lhsT=ones_sb[:], rhs=acc[:, :],
                         start=True, stop=True)
        total = smallp.tile([1, TI * (NJC + 1)], f32, tag="tot")
        nc.vector.tensor_copy(total[:], tot_ps[:])
        nc.sync.dma_start(out=out_d[:, :], in_=total[:])

    nc.compile()
    return nc


def _get_nc():
    if "nc" not in _cache:
        _cache["nc"] = _build()
    return _cache["nc"]


def _prep_inputs(wsi, omic):
    W = np.asarray(wsi, dtype=np.float32)[:, 0, :].astype(np.float64)
    O = np.asarray(omic, dtype=np.float32)[:, 0, :].astype(np.float64)
    Wn = (W / np.maximum(np.linalg.norm(W, axis=1, keepdims=True), 1e-30))
    On = (O / np.maximum(np.linalg.norm(O, axis=1, keepdims=True), 1e-30))
    Wn = Wn.astype(ml_dtypes.bfloat16)
    On = On.astype(ml_dtypes.bfloat16)
    ident = np.eye(P, dtype=np.float32)

    in_maps = []
    for c in range(NCORES):
        Wc = Wn[c * ROWS:(c + 1) * ROWS]  # [512, 1024]
        # wt[k, (t*ND + d)*P + m] = Wc[t*P + m, d*P + k]
        wt = np.ascontiguousarray(
            Wc.reshape(TI, P, ND, P).transpose(3, 0, 2, 1).reshape(P, TI * ND * P))
        # column rotation: permuted col j' <-> original O row (j' + 512c) % N
        Operm = np.roll(On, -ROWS * c, axis=0)
        # ot[k, (jc*ND + d)*NJ + n] = Operm[jc*NJ + n, d*P + k]
        ot = np.ascontiguousarray(
            Operm.reshape(NJC, NJ, ND, P).transpose(3, 0, 2, 1)
            .reshape(P, NJC * ND * NJ))
        in_maps.append({"wt": wt, "ot": ot, "id": ident})
    return in_maps


def kernel(wsi_embeddings, omic_embeddings):
    from concourse.bass_utils import run_bass_kernel_spmd

    nc = _get_nc()
    in_maps = _prep_inputs(wsi_embeddings, omic_embeddings)
    res = run_bass_kernel_spmd(nc, in_maps, list(range(NCORES)))
    grand = 0.0
    for c in range(NCORES):
        grand += res.results[c]["out"].astype(np.float64).sum()
    return np.float32(grand / (float(N) * float(N)))
